# revision 13
# baseline (speedup 1.0000x reference)
"""GQA attention (RoPE + ALiBi + causal) Bass kernel for Trainium2, 8 NeuronCores.

Sharding: core (b, g) = batch b in {0,1} x kv-group g in {0..3}; each core computes
its 4 query heads' attention for its batch and a partial output projection
(row-parallel wo); host sums the 4 group partials per batch.

All matmul operands are fp32r (empirically the fastest per-matmul cadence on
TRN2 at 512-wide moving dim; 16-bit operands pay ~+32ns/matmul, and gpsimd
cast-DMAs from fp16 DRAM are too slow to feed phase 1). The output partial is
written back as fp16 to halve writeback traffic.

  Phase 1 (per 512-q window): Q/K/V projections (6 PSUM accumulators over one
  streamed pass of xT; weight-chunk and x-tile cast-DMAs interleaved in
  consumption order on the gpsimd queue), PSUM->SBUF copies on ACT, per-window
  RoPE (DVE) and V transpose (PE) so nothing piles up at the phase boundary.
  Phase 2: per (window, head): scoresT = K^T Q (PSUM), P = exp(scale*scores
  + bias) with per-head-slot exp widths {128,256,256,512}: heads are ordered by
  descending ALiBi slope within each GQA group, so later slots (smaller slopes)
  tolerate coarser per-chunk bias recentering - fewer, wider ACT instructions.
  The -slope*q half of ALiBi cancels in softmax; the per-kv half plus the
  chunk-recentering constant comes from a host-built bias table indexed by
  (slot, chunkpos - kvtile). Diagonal chunks get a causal 0/1 mask multiply.
  Then outT += V_u^T P and den += ones^T P (PSUM accumulate); attn =
  outT * recip(den); each window's output-projection matmuls are interleaved
  as PE filler into the next window's attention loop, front-loaded onto the
  exp-heavy slot-0 head.
"""
import math
from contextlib import ExitStack

import numpy as np

import concourse.bass as bass
import concourse.bacc as bacc
import concourse.tile as tile
from concourse import mybir
from concourse.bass_utils import run_bass_kernel_spmd

F32 = mybir.dt.float32
F32R = mybir.dt.float32r
F16 = mybir.dt.float16

B, S, D = 2, 2048, 2048
H, KV, HD, REP = 16, 4, 128, 4
NH = 4                     # heads per core
NW = S // 512              # q-windows
ND = D // 128              # d_in tiles
NU = S // 128              # kv tiles
SCALE = 1.0 / math.sqrt(HD)

# per-head-slot exp chunk width and bias recentering constant; slot 0 holds the
# steepest ALiBi slope of the core's group (heads are slope-descending within a
# group), so it gets the finest recentering. Range check (worst slope s per
# slot, scores*scale ~ N(0,1)):
#   exp arg in [score - s*C, score + s*(W-1-C)] -> within fp32's e^{+-87} for
#   (W,C,s) = (128,96,.707), (256,127,.5), (256,127,.354), (512,255,.25).
EXP_W = [128, 256, 256, 512]
EXP_C = [96.0, 127.0, 127.0, 255.0]
NBM = 19                   # bias cols per head slot: m = gridpos - kvtile in [-3, 15]


def build():
    nc = bacc.Bacc(None)
    xT_d = nc.dram_tensor("xT", [D, S], F32R, kind="ExternalInput")
    wq_d = nc.dram_tensor("wqT", [D, NH * HD], F32R, kind="ExternalInput")
    wk_d = nc.dram_tensor("wkT", [D, HD], F32R, kind="ExternalInput")
    wv_d = nc.dram_tensor("wvT", [D, HD], F32R, kind="ExternalInput")
    wo_d = nc.dram_tensor("woT", [NH * HD, D], F32R, kind="ExternalInput")
    cosF_d = nc.dram_tensor("cosF", [128, S], F32R, kind="ExternalInput")
    sinF_d = nc.dram_tensor("sinF", [128, S], F32R, kind="ExternalInput")
    biasb_d = nc.dram_tensor("biasb", [128, NH * NBM], F32, kind="ExternalInput")
    cmask_d = nc.dram_tensor("cmask", [128, 128], F32R, kind="ExternalInput")
    ident_d = nc.dram_tensor("ident", [128, 128], F32R, kind="ExternalInput")
    ones_d = nc.dram_tensor("ones", [128, 128], F32R, kind="ExternalInput")
    part_d = nc.dram_tensor("part", [S, D], F16, kind="ExternalOutput")

    PSUM = bass.MemorySpace.PSUM

    with tile.TileContext(nc) as tc:
        with ExitStack() as ctx:
            consts = ctx.enter_context(tc.tile_pool(name="consts", bufs=1))
            persist = ctx.enter_context(tc.tile_pool(name="persist", bufs=1))

            cosF = consts.tile([128, S], F32R, tag="cosF")
            sinF = consts.tile([128, S], F32R, tag="sinF")
            biasb = consts.tile([128, NH * NBM], F32, tag="biasb")
            cmask = consts.tile([128, 128], F32R, tag="cmask")
            ident = consts.tile([128, 128], F32R, tag="ident")
            ones = consts.tile([128, 128], F32R, tag="ones")

            qT = [persist.tile([128, S], F32R, tag=f"qT{h}", name=f"qT{h}")
                  for h in range(NH)]
            kT = persist.tile([128, S], F32R, tag="kT")
            vnat = persist.tile([128, S], F32R, tag="vnat")
            attn = [persist.tile([128, S], F32R, tag=f"attn{h}", name=f"attn{h}")
                    for h in range(NH)]

            # ---------------- phase 1: Q/K/V projections (+ per-window RoPE) -----
            with tc.tile_pool(name="wqkv", bufs=1) as wpool, \
                 tc.tile_pool(name="xsl", bufs=10) as xpool, \
                 tc.tile_pool(name="vtmp", bufs=1) as vpool, \
                 tc.tile_pool(name="rope", bufs=3) as rp, \
                 tc.tile_pool(name="pps", bufs=1, space=PSUM) as pps, \
                 tc.tile_pool(name="tpp", bufs=1, space=PSUM) as tpp:
                wq_sb = wpool.tile([128, ND, NH * HD], F32R, tag="wq")
                wk_sb = wpool.tile([128, ND, HD], F32R, tag="wk")
                wv_sb = wpool.tile([128, ND, HD], F32R, tag="wv")
                wqr = wq_d.rearrange("(t p) o -> p t o", p=128)
                wkr = wk_d.rearrange("(t p) o -> p t o", p=128)
                wvr = wv_d.rearrange("(t p) o -> p t o", p=128)

                xtiles = {}

                def issue_x(w, d):
                    xs = xpool.tile([128, 512], F32R, tag="x", name="xs")
                    nc.sync.dma_start(
                        xs[:], xT_d[d * 128:(d + 1) * 128, w * 512:(w + 1) * 512])
                    xtiles[(w, d)] = xs

                # weight chunks on gpsimd (groups of 2 d-tiles: fast first
                # arrival, JIT thereafter); window-0 x prefetch on sync ahead
                # of the rope tables
                for g2 in range(ND // 2):
                    dsl = slice(2 * g2, 2 * g2 + 2)
                    nc.gpsimd.dma_start(wk_sb[:, dsl, :], wkr[:, dsl, :])
                    nc.gpsimd.dma_start(wv_sb[:, dsl, :], wvr[:, dsl, :])
                    nc.gpsimd.dma_start(wq_sb[:, dsl, :], wqr[:, dsl, :])
                for d in range(ND):
                    issue_x(0, d)
                # rope tables + small consts behind the w0 x prefetch
                # (first needed ~26us in)
                nc.sync.dma_start(cosF[:], cosF_d[:])
                nc.sync.dma_start(sinF[:], sinF_d[:])
                nc.sync.dma_start(biasb[:], biasb_d[:])
                nc.sync.dma_start(cmask[:], cmask_d[:])
                nc.sync.dma_start(ident[:], ident_d[:])
                nc.sync.dma_start(ones[:], ones_d[:])
                vT = vpool.tile([128, S], F32R, tag="vT")

                for w in range(NW):
                    sl = slice(w * 512, (w + 1) * 512)
                    pq = [pps.tile([128, 512], F32, tag=f"pq{h}", name=f"pq{h}")
                          for h in range(NH)]
                    pk = pps.tile([128, 512], F32, tag="pk", name="pk")
                    pv = pps.tile([128, 512], F32, tag="pv", name="pv")
                    for d in range(ND):
                        if w > 0:
                            issue_x(w, d)
                        xs = xtiles.pop((w, d))
                        st, sp = (d == 0), (d == ND - 1)
                        nc.tensor.matmul(pk[:], wk_sb[:, d, :], xs[:], start=st, stop=sp)
                        nc.tensor.matmul(pv[:], wv_sb[:, d, :], xs[:], start=st, stop=sp)
                        for h in range(NH):
                            nc.tensor.matmul(pq[h][:], wq_sb[:, d, h * 128:(h + 1) * 128],
                                             xs[:], start=st, stop=sp)
                    # PSUM->SBUF copies split across ACT and DVE
                    nc.scalar.copy(kT[:, sl], pk[:])
                    nc.vector.tensor_copy(vT[:, sl], pv[:])
                    nc.scalar.copy(qT[0][:, sl], pq[0][:])
                    nc.scalar.copy(qT[1][:, sl], pq[1][:])
                    nc.vector.tensor_copy(qT[2][:, sl], pq[2][:])
                    nc.vector.tensor_copy(qT[3][:, sl], pq[3][:])

                    # RoPE on this window's q/k slices: out = cosF*z + sinF*swap(z)
                    for tgt in [kT] + qT:
                        qb = rp.tile([128, 512], F32R, tag="qb", name="qb")
                        nc.sync.dma_start(qb[0:64, :], tgt[64:128, sl])
                        nc.sync.dma_start(qb[64:128, :], tgt[0:64, sl])
                        t1 = rp.tile([128, 512], F32R, tag="t1", name="t1")
                        nc.vector.tensor_mul(t1[:], tgt[:, sl], cosF[:, sl])
                        nc.vector.tensor_mul(qb[:], qb[:], sinF[:, sl])
                        nc.vector.tensor_add(tgt[:, sl], t1[:], qb[:])

                    # V transpose for this window's 4 kv tiles:
                    # vT [hd, s] -> vnat [s(part), hd]
                    for u in range(4 * w, 4 * w + 4):
                        tp = tpp.tile([128, 128], F32R, tag=f"tp{u % 2}",
                                      name=f"tp{u}")
                        nc.tensor.transpose(tp[:], vT[:, u * 128:(u + 1) * 128],
                                            ident[:])
                        nc.scalar.copy(vnat[:, u * 128:(u + 1) * 128], tp[:])

            # ---------------- phase 2: attention + output projection ------------
            with tc.tile_pool(name="sp", bufs=3, space=PSUM) as sp, \
                 tc.tile_pool(name="dp", bufs=2, space=PSUM) as dp, \
                 tc.tile_pool(name="op", bufs=2, space=PSUM) as op, \
                 tc.tile_pool(name="ojp", bufs=1, space=PSUM) as ojp, \
                 tc.tile_pool(name="Pp", bufs=8) as Pp, \
                 tc.tile_pool(name="ep", bufs=4) as ep, \
                 tc.tile_pool(name="wop", bufs=1) as wop, \
                 tc.tile_pool(name="ostg", bufs=6) as ostg:
                wo_sb = wop.tile([128, NH, D], F32R, tag="wo")
                nc.gpsimd.dma_start(wo_sb[:], wo_d.rearrange("(h p) o -> p h o", p=128))

                filler_q = []

                def emit_fillers(n):
                    for _ in range(n):
                        if not filler_q:
                            return
                        filler_q.pop(0)()

                def make_unit(w_, mq_, dwin_):
                    def unit():
                        m_ = 4 * w_ + mq_
                        po = ojp.tile([128, 512], F32, tag="oj",
                                      name=f"po{m_}_{dwin_}")
                        for h_ in range(NH):
                            nc.tensor.matmul(
                                po[:],
                                attn[h_][:, m_ * 128:(m_ + 1) * 128],
                                wo_sb[:, h_, dwin_ * 512:(dwin_ + 1) * 512],
                                start=(h_ == 0), stop=(h_ == NH - 1))
                        so = ostg.tile([128, 512], F16, tag="so", name="so")
                        nc.vector.tensor_copy(so[:], po[:])
                        nc.sync.dma_start(
                            part_d[m_ * 128:(m_ + 1) * 128,
                                   dwin_ * 512:(dwin_ + 1) * 512], so[:])
                    return unit

                ucount = 0
                for w in range(NW):
                    qsl = slice(w * 512, (w + 1) * 512)
                    U = 4 * (w + 1)
                    for h in range(NH):
                        o_ps = op.tile([128, 512], F32, tag="o", name=f"o{w}_{h}")
                        d_ps = dp.tile([128, 512], F32, tag="den", name=f"d{w}_{h}")
                        pend = None
                        for u in range(U):
                            i0 = max(0, u - 4 * w)
                            n0 = 128 * i0
                            s_ps = sp.tile([128, 512], F32, tag="s", name="s")
                            nc.tensor.matmul(
                                s_ps[:, n0:512],
                                kT[:, u * 128:(u + 1) * 128],
                                qT[h][:, w * 512 + n0:(w + 1) * 512],
                                start=True, stop=True)
                            Pt = Pp.tile([128, 512], F32R, tag="P", name="P")
                            # exp pieces on a fixed per-slot q-grid (grid
                            # position independent of u so the dropped per-q
                            # ALiBi term is consistent across kv tiles);
                            # diagonal tiles clip the first piece at n0.
                            # bias col m = grid position (128-units) - kv tile
                            for gs in range(0, 512, EXP_W[h]):
                                pcs = max(gs, n0)
                                pce = gs + EXP_W[h]
                                if pcs >= pce:
                                    continue
                                m = 4 * w + gs // 128 - u + 3
                                nc.scalar.activation(
                                    Pt[:, pcs:pce],
                                    s_ps[:, pcs:pce],
                                    mybir.ActivationFunctionType.Exp,
                                    bias=biasb[:, h * NBM + m:h * NBM + m + 1],
                                    scale=SCALE)
                            if u >= 4 * w:
                                # diagonal kv tile: causal 0/1 mask on its block
                                nc.vector.tensor_mul(Pt[:, n0:n0 + 128],
                                                     Pt[:, n0:n0 + 128], cmask[:])
                            ucount += 1
                            # front-load PE filler onto the exp-heavy slot-0
                            # head; later slots are self-paced
                            if h == 0 or ucount % 2 == 0:
                                emit_fillers(1)
                            if pend is not None:
                                pPt, pn0, pu = pend
                                nc.tensor.matmul(o_ps[:, pn0:512],
                                                 vnat[:, pu * 128:(pu + 1) * 128],
                                                 pPt[:, pn0:512],
                                                 start=(pu == 0), stop=False)
                                nc.tensor.matmul(d_ps[:, pn0:512], ones[:],
                                                 pPt[:, pn0:512],
                                                 start=(pu == 0), stop=False)
                            pend = (Pt, n0, u)
                        pPt, pn0, pu = pend
                        nc.tensor.matmul(o_ps[:, pn0:512],
                                         vnat[:, pu * 128:(pu + 1) * 128],
                                         pPt[:, pn0:512], start=(pu == 0), stop=True)
                        nc.tensor.matmul(d_ps[:, pn0:512], ones[:],
                                         pPt[:, pn0:512], start=(pu == 0), stop=True)
                        rec = ep.tile([128, 512], F32, tag="rec", name="rec")
                        nc.vector.reciprocal_approx_fast(rec[:], d_ps[:])
                        nc.vector.tensor_mul(attn[h][:, qsl], o_ps[:], rec[:])

                    # enqueue this window's output projection as PE filler
                    # for the next window's attention loop
                    for mq in range(4):
                        for dwin in range(4):
                            filler_q.append(make_unit(w, mq, dwin))

                emit_fillers(len(filler_q))
    nc.finalize()
    return nc


_NC_CACHE = {}


def _get_nc():
    if "nc" not in _NC_CACHE:
        _NC_CACHE["nc"] = build()
    return _NC_CACHE["nc"]


def _host_prep(x, alibi_bias, wq, wk, wv, wo):
    """Build per-core input maps (shard + transpose + rope tables + bias tables)."""
    x = np.asarray(x, np.float32)
    alibi_bias = np.asarray(alibi_bias, np.float32)
    wq = np.asarray(wq, np.float32)
    wk = np.asarray(wk, np.float32)
    wv = np.asarray(wv, np.float32)
    wo = np.asarray(wo, np.float32)

    slopes = alibi_bias[0, :, 0, 1].copy()        # [H]; alibi[0,h,0,1] = slope_h

    inv_freq = 1.0 / (10000.0 ** (np.arange(0, HD, 2, dtype=np.float32) / HD))
    t = np.arange(S, dtype=np.float32)
    freqs = np.outer(t, inv_freq)                 # [S, 64]
    cos = np.cos(freqs).astype(np.float32).T      # [64, S]
    sin = np.sin(freqs).astype(np.float32).T
    cosF = np.ascontiguousarray(np.concatenate([cos, cos], 0))     # [128, S]
    sinF = np.ascontiguousarray(np.concatenate([-sin, sin], 0))

    perm = np.concatenate([np.arange(0, HD, 2), np.arange(1, HD, 2)])
    p_ar = np.arange(128, dtype=np.float32)
    cmask = (p_ar[:, None] <= p_ar[None, :]).astype(np.float32)
    ident = np.eye(128, dtype=np.float32)
    ones = np.ones((128, 128), np.float32)

    xTs = [np.ascontiguousarray(x[b].T) for b in range(B)]
    in_maps = []
    for core in range(8):
        b, g = divmod(core, KV)
        wq_g = wq[4 * g * HD:(4 * g + 4) * HD].reshape(NH, HD, D)[:, perm, :]
        wqT = np.ascontiguousarray(wq_g.reshape(NH * HD, D).T)
        wkT = np.ascontiguousarray(wk[g * HD:(g + 1) * HD][perm].T)
        wvT = np.ascontiguousarray(wv[g * HD:(g + 1) * HD].T)
        woT = np.ascontiguousarray(wo[:, 4 * g * HD:(4 * g + 4) * HD].T)
        biasb = np.zeros((128, NH * NBM), np.float32)
        for h in range(NH):
            sl = slopes[4 * g + h]
            for mi in range(NBM):
                m = mi - 3   # m = gridpos - kvtile; bias = slope*(j - i_gs - C)
                biasb[:, h * NBM + mi] = sl * (p_ar - 128.0 * m - EXP_C[h])
        in_maps.append({
            "xT": xTs[b], "wqT": wqT, "wkT": wkT, "wvT": wvT, "woT": woT,
            "cosF": cosF, "sinF": sinF, "biasb": biasb, "cmask": cmask,
            "ident": ident, "ones": ones,
        })
    return in_maps


def kernel(x, mask, alibi_bias, wq, wk, wv, wo, _trace=False, _trace_kwargs=None):
    nc = _get_nc()
    in_maps = _host_prep(x, alibi_bias, wq, wk, wv, wo)
    res = run_bass_kernel_spmd(nc, in_maps, list(range(8)), trace=_trace,
                               **(_trace_kwargs or {}))
    parts = [np.asarray(res.results[c]["part"], np.float32) for c in range(8)]
    out = np.stack([
        parts[0] + parts[1] + parts[2] + parts[3],
        parts[4] + parts[5] + parts[6] + parts[7],
    ]).astype(np.float32)
    if _trace:
        return out, res
    return out


# revision 14
# speedup vs baseline: 1.4668x; 1.4668x over previous
"""GQA attention (RoPE + ALiBi + causal) Bass kernel for Trainium2, 8 NeuronCores.

Sharding: core (b, g) = batch b in {0,1} x kv-group g in {0..3}; each core computes
its 4 query heads' attention for its batch and a partial output projection
(row-parallel wo); host sums the 4 group partials per batch.

Dataflow is 16-bit on the PE: fp16 for x/weights/Q/K/attn, bf16 for P and V
(bf16's fp32-size exponent carries the ALiBi recentering range). 16-bit
matmuls pay ~+32ns each over fp32r but halve PE power, which avoids most of
the chip's activity throttling (fp32r at full duty loses more to the 0.5x
util cap than the cadence penalty costs), and halve DMA traffic and SBUF.

  Phase 1 (per 512-q window): Q/K/V projections (6 PSUM accumulators over one
  streamed pass of xT). The first weight chunks and the window-0 x tiles go on
  the sync hwdge queue (earliest to start issuing); later weight chunks stream
  from gpsimd. PSUM->SBUF copies split across ACT/DVE (all-ACT for the last
  window so the DVE's RoPE backlog at the phase boundary is minimal),
  per-window RoPE (DVE) and V transpose (PE).
  Phase 2: per (window, head): scoresT = K^T Q (PSUM), P = exp(scale*scores
  + bias) with per-head-slot exp widths {128,256,256,512}: heads are ordered by
  descending ALiBi slope within each GQA group, so later slots (smaller slopes)
  tolerate coarser per-chunk bias recentering - fewer, wider ACT instructions.
  The -slope*q half of ALiBi cancels in softmax; the per-kv half plus the
  chunk-recentering constant comes from a host-built bias table indexed by
  (slot, chunkpos - kvtile). Diagonal chunks get a causal 0/1 mask multiply.
  Then outT += V_u^T P and den += ones^T P (PSUM accumulate); attn =
  outT * recip(den); each window's output-projection matmuls are interleaved
  as PE filler into the next window's attention loop, front-loaded onto the
  exp-heavy slot-0 head. Window 0 runs its heads cheapest-exp-first while the
  filler queue is still empty.
"""
import math
from contextlib import ExitStack

import numpy as np
import ml_dtypes

import concourse.bass as bass
import concourse.bacc as bacc
import concourse.tile as tile
from concourse import mybir
from concourse.bass_utils import run_bass_kernel_spmd

F32 = mybir.dt.float32
F16 = mybir.dt.float16
BF16 = mybir.dt.bfloat16

B, S, D = 2, 2048, 2048
H, KV, HD, REP = 16, 4, 128, 4
NH = 4                     # heads per core
NW = S // 512              # q-windows
ND = D // 128              # d_in tiles
NU = S // 128              # kv tiles
SCALE = 1.0 / math.sqrt(HD)

# per-head-slot exp chunk width and bias recentering constant; slot 0 holds the
# steepest ALiBi slope of the core's group (heads are slope-descending within a
# group), so it gets the finest recentering. Range check (worst slope s per
# slot, scores*scale ~ N(0,1), P in bf16 with fp32-range exponent):
#   exp arg in [score - s*C, score + s*(W-1-C)] -> within e^{+-87} for
#   (W,C,s) = (128,96,.707), (256,127,.5), (256,127,.354), (512,255,.25).
EXP_W = [128, 256, 256, 512]
EXP_C = [96.0, 127.0, 127.0, 255.0]
NBM = 19                   # bias cols per head slot: m = gridpos - kvtile in [-3, 15]


def build():
    nc = bacc.Bacc(None)
    xT_d = nc.dram_tensor("xT", [D, S], F16, kind="ExternalInput")
    wq_d = nc.dram_tensor("wqT", [D, NH * HD], F16, kind="ExternalInput")
    wk_d = nc.dram_tensor("wkT", [D, HD], F16, kind="ExternalInput")
    wv_d = nc.dram_tensor("wvT", [D, HD], F16, kind="ExternalInput")
    wo_d = nc.dram_tensor("woT", [NH * HD, D], F16, kind="ExternalInput")
    cosF_d = nc.dram_tensor("cosF", [128, S], F16, kind="ExternalInput")
    sinF_d = nc.dram_tensor("sinF", [128, S], F16, kind="ExternalInput")
    biasb_d = nc.dram_tensor("biasb", [128, NH * NBM], F32, kind="ExternalInput")
    cmask_d = nc.dram_tensor("cmask", [128, 128], BF16, kind="ExternalInput")
    ident_d = nc.dram_tensor("ident", [128, 128], BF16, kind="ExternalInput")
    ones_d = nc.dram_tensor("ones", [128, 128], BF16, kind="ExternalInput")
    part_d = nc.dram_tensor("part", [S, D], F16, kind="ExternalOutput")

    PSUM = bass.MemorySpace.PSUM

    with tile.TileContext(nc) as tc:
        with ExitStack() as ctx:
            consts = ctx.enter_context(tc.tile_pool(name="consts", bufs=1))
            persist = ctx.enter_context(tc.tile_pool(name="persist", bufs=1))

            cosF = consts.tile([128, S], F16, tag="cosF")
            sinF = consts.tile([128, S], F16, tag="sinF")
            biasb = consts.tile([128, NH * NBM], F32, tag="biasb")
            cmask = consts.tile([128, 128], BF16, tag="cmask")
            ident = consts.tile([128, 128], BF16, tag="ident")
            ones = consts.tile([128, 128], BF16, tag="ones")

            qT = [persist.tile([128, S], F16, tag=f"qT{h}", name=f"qT{h}")
                  for h in range(NH)]
            kT = persist.tile([128, S], F16, tag="kT")
            vnat = persist.tile([128, S], BF16, tag="vnat")
            attn = [persist.tile([128, S], F16, tag=f"attn{h}", name=f"attn{h}")
                    for h in range(NH)]

            # ---------------- phase 1: Q/K/V projections (+ per-window RoPE) -----
            with tc.tile_pool(name="wqkv", bufs=1) as wpool, \
                 tc.tile_pool(name="xsl", bufs=10) as xpool, \
                 tc.tile_pool(name="vtmp", bufs=1) as vpool, \
                 tc.tile_pool(name="rope", bufs=3) as rp, \
                 tc.tile_pool(name="pps", bufs=1, space=PSUM) as pps, \
                 tc.tile_pool(name="tpp", bufs=1, space=PSUM) as tpp:
                wq_sb = wpool.tile([128, ND, NH * HD], F16, tag="wq")
                wk_sb = wpool.tile([128, ND, HD], F16, tag="wk")
                wv_sb = wpool.tile([128, ND, HD], F16, tag="wv")
                wqr = wq_d.rearrange("(t p) o -> p t o", p=128)
                wkr = wk_d.rearrange("(t p) o -> p t o", p=128)
                wvr = wv_d.rearrange("(t p) o -> p t o", p=128)

                xtiles = {}

                def issue_x(w, d):
                    xs = xpool.tile([128, 512], F16, tag="x", name="xs")
                    nc.sync.dma_start(
                        xs[:], xT_d[d * 128:(d + 1) * 128, w * 512:(w + 1) * 512])
                    xtiles[(w, d)] = xs

                # critical path on the sync hwdge queue (first to issue):
                # d0-1 weight chunks, then window-0 x tiles; remaining weight
                # chunks stream JIT from gpsimd; rope tables/consts follow the
                # w0 x prefetch (first needed ~25us in)
                d01 = slice(0, 2)
                nc.sync.dma_start(wk_sb[:, d01, :], wkr[:, d01, :])
                nc.sync.dma_start(wv_sb[:, d01, :], wvr[:, d01, :])
                nc.sync.dma_start(wq_sb[:, d01, :], wqr[:, d01, :])
                for g2 in range(1, ND // 2):
                    dsl = slice(2 * g2, 2 * g2 + 2)
                    nc.gpsimd.dma_start(wk_sb[:, dsl, :], wkr[:, dsl, :])
                    nc.gpsimd.dma_start(wv_sb[:, dsl, :], wvr[:, dsl, :])
                    nc.gpsimd.dma_start(wq_sb[:, dsl, :], wqr[:, dsl, :])
                for d in range(ND):
                    issue_x(0, d)
                nc.sync.dma_start(cosF[:], cosF_d[:])
                nc.sync.dma_start(sinF[:], sinF_d[:])
                nc.sync.dma_start(biasb[:], biasb_d[:])
                nc.sync.dma_start(cmask[:], cmask_d[:])
                nc.sync.dma_start(ident[:], ident_d[:])
                nc.sync.dma_start(ones[:], ones_d[:])
                vT = vpool.tile([128, S], BF16, tag="vT")

                for w in range(NW):
                    sl = slice(w * 512, (w + 1) * 512)
                    pq = [pps.tile([128, 512], F32, tag=f"pq{h}", name=f"pq{h}")
                          for h in range(NH)]
                    pk = pps.tile([128, 512], F32, tag="pk", name="pk")
                    pv = pps.tile([128, 512], F32, tag="pv", name="pv")
                    for d in range(ND):
                        if w > 0:
                            issue_x(w, d)
                        xs = xtiles.pop((w, d))
                        st, sp = (d == 0), (d == ND - 1)
                        nc.tensor.matmul(pk[:], wk_sb[:, d, :], xs[:], start=st, stop=sp)
                        nc.tensor.matmul(pv[:], wv_sb[:, d, :], xs[:], start=st, stop=sp)
                        for h in range(NH):
                            nc.tensor.matmul(pq[h][:], wq_sb[:, d, h * 128:(h + 1) * 128],
                                             xs[:], start=st, stop=sp)
                    # PSUM->SBUF copies split across ACT and DVE; the last
                    # window goes all-ACT so the DVE backlog at the phase
                    # boundary is only the rope itself
                    last = (w == NW - 1)
                    nc.scalar.copy(kT[:, sl], pk[:])
                    nc.scalar.copy(qT[0][:, sl], pq[0][:])
                    nc.scalar.copy(qT[1][:, sl], pq[1][:])
                    if last:
                        nc.scalar.copy(vT[:, sl], pv[:])
                        nc.scalar.copy(qT[2][:, sl], pq[2][:])
                        nc.scalar.copy(qT[3][:, sl], pq[3][:])
                    else:
                        nc.vector.tensor_copy(vT[:, sl], pv[:])
                        nc.vector.tensor_copy(qT[2][:, sl], pq[2][:])
                        nc.vector.tensor_copy(qT[3][:, sl], pq[3][:])

                    # RoPE on this window's q/k slices: out = cosF*z + sinF*swap(z)
                    for tgt in [kT] + qT:
                        qb = rp.tile([128, 512], F16, tag="qb", name="qb")
                        nc.sync.dma_start(qb[0:64, :], tgt[64:128, sl])
                        nc.sync.dma_start(qb[64:128, :], tgt[0:64, sl])
                        t1 = rp.tile([128, 512], F16, tag="t1", name="t1")
                        nc.vector.tensor_mul(t1[:], tgt[:, sl], cosF[:, sl])
                        nc.vector.tensor_mul(qb[:], qb[:], sinF[:, sl])
                        nc.vector.tensor_add(tgt[:, sl], t1[:], qb[:])

                    # V transpose for this window's 4 kv tiles:
                    # vT [hd, s] -> vnat [s(part), hd]
                    for u in range(4 * w, 4 * w + 4):
                        tp = tpp.tile([128, 128], BF16, tag=f"tp{u % 2}",
                                      name=f"tp{u}")
                        nc.tensor.transpose(tp[:], vT[:, u * 128:(u + 1) * 128],
                                            ident[:])
                        nc.scalar.copy(vnat[:, u * 128:(u + 1) * 128], tp[:])

            # ---------------- phase 2: attention + output projection ------------
            with tc.tile_pool(name="sp", bufs=3, space=PSUM) as sp, \
                 tc.tile_pool(name="dp", bufs=1, space=PSUM) as dp, \
                 tc.tile_pool(name="op", bufs=2, space=PSUM) as op, \
                 tc.tile_pool(name="ojp", bufs=2, space=PSUM) as ojp, \
                 tc.tile_pool(name="Pp", bufs=8) as Pp, \
                 tc.tile_pool(name="ep", bufs=4) as ep, \
                 tc.tile_pool(name="wop", bufs=1) as wop, \
                 tc.tile_pool(name="ostg", bufs=6) as ostg:
                wo_sb = wop.tile([128, NH, D], F16, tag="wo")
                nc.gpsimd.dma_start(wo_sb[:], wo_d.rearrange("(h p) o -> p h o", p=128))

                filler_q = []

                def emit_fillers(n, eng="v"):
                    for _ in range(n):
                        if not filler_q:
                            return
                        filler_q.pop(0)(eng)

                def make_unit(w_, mq_, dwin_):
                    def unit(eng):
                        m_ = 4 * w_ + mq_
                        po = ojp.tile([128, 512], F32, tag="oj",
                                      name=f"po{m_}_{dwin_}")
                        for h_ in range(NH):
                            nc.tensor.matmul(
                                po[:],
                                attn[h_][:, m_ * 128:(m_ + 1) * 128],
                                wo_sb[:, h_, dwin_ * 512:(dwin_ + 1) * 512],
                                start=(h_ == 0), stop=(h_ == NH - 1))
                        so = ostg.tile([128, 512], F16, tag="so", name="so")
                        if eng == "v":
                            nc.vector.tensor_copy(so[:], po[:])
                        else:
                            nc.scalar.copy(so[:], po[:])
                        nc.sync.dma_start(
                            part_d[m_ * 128:(m_ + 1) * 128,
                                   dwin_ * 512:(dwin_ + 1) * 512], so[:])
                    return unit

                ucount = 0
                for w in range(NW):
                    qsl = slice(w * 512, (w + 1) * 512)
                    U = 4 * (w + 1)
                    # window 0 runs cheapest-exp heads first (no filler supply
                    # yet, so let ACT race ahead of the PE)
                    horder = [3, 2, 1, 0] if w == 0 else [0, 1, 2, 3]
                    for h in horder:
                        o_ps = op.tile([128, 512], F32, tag="o", name=f"o{w}_{h}")
                        d_ps = dp.tile([128, 512], F32, tag="den", name=f"d{w}_{h}")
                        pend = None
                        for u in range(U):
                            i0 = max(0, u - 4 * w)
                            n0 = 128 * i0
                            s_ps = sp.tile([128, 512], F32, tag="s", name="s")
                            nc.tensor.matmul(
                                s_ps[:, n0:512],
                                kT[:, u * 128:(u + 1) * 128],
                                qT[h][:, w * 512 + n0:(w + 1) * 512],
                                start=True, stop=True)
                            Pt = Pp.tile([128, 512], BF16, tag="P", name="P")
                            # exp pieces on a fixed per-slot q-grid (grid
                            # position independent of u so the dropped per-q
                            # ALiBi term is consistent across kv tiles);
                            # diagonal tiles clip the first piece at n0.
                            # bias col m = grid position (128-units) - kv tile
                            for gs in range(0, 512, EXP_W[h]):
                                pcs = max(gs, n0)
                                pce = gs + EXP_W[h]
                                if pcs >= pce:
                                    continue
                                m = 4 * w + gs // 128 - u + 3
                                nc.scalar.activation(
                                    Pt[:, pcs:pce],
                                    s_ps[:, pcs:pce],
                                    mybir.ActivationFunctionType.Exp,
                                    bias=biasb[:, h * NBM + m:h * NBM + m + 1],
                                    scale=SCALE)
                            if u >= 4 * w:
                                # diagonal kv tile: causal 0/1 mask on its block
                                nc.vector.tensor_mul(Pt[:, n0:n0 + 128],
                                                     Pt[:, n0:n0 + 128], cmask[:])
                            ucount += 1
                            # front-load PE filler onto the exp-heavy slot-0
                            # head; later slots are self-paced
                            if h == 0 or ucount % 2 == 0:
                                emit_fillers(1)
                            if pend is not None:
                                pPt, pn0, pu = pend
                                nc.tensor.matmul(o_ps[:, pn0:512],
                                                 vnat[:, pu * 128:(pu + 1) * 128],
                                                 pPt[:, pn0:512],
                                                 start=(pu == 0), stop=False)
                                nc.tensor.matmul(d_ps[:, pn0:512], ones[:],
                                                 pPt[:, pn0:512],
                                                 start=(pu == 0), stop=False)
                            pend = (Pt, n0, u)
                        pPt, pn0, pu = pend
                        nc.tensor.matmul(o_ps[:, pn0:512],
                                         vnat[:, pu * 128:(pu + 1) * 128],
                                         pPt[:, pn0:512], start=(pu == 0), stop=True)
                        nc.tensor.matmul(d_ps[:, pn0:512], ones[:],
                                         pPt[:, pn0:512], start=(pu == 0), stop=True)
                        rec = ep.tile([128, 512], F32, tag="rec", name="rec")
                        nc.vector.reciprocal_approx_fast(rec[:], d_ps[:])
                        nc.vector.tensor_mul(attn[h][:, qsl], o_ps[:], rec[:])

                    # enqueue this window's output projection as PE filler
                    # for the next window's attention loop
                    for mq in range(4):
                        for dwin in range(4):
                            filler_q.append(make_unit(w, mq, dwin))

                # drain remaining units; their staging copies go on ACT, which
                # is idle once the exps are done
                emit_fillers(len(filler_q), eng="s")
    nc.finalize()
    return nc


_NC_CACHE = {}


def _get_nc():
    if "nc" not in _NC_CACHE:
        _NC_CACHE["nc"] = build()
    return _NC_CACHE["nc"]


def _host_prep(x, alibi_bias, wq, wk, wv, wo):
    """Build per-core input maps (shard + transpose + rope tables + bias tables)."""
    x = np.asarray(x, np.float32)
    alibi_bias = np.asarray(alibi_bias, np.float32)
    wq = np.asarray(wq, np.float32)
    wk = np.asarray(wk, np.float32)
    wv = np.asarray(wv, np.float32)
    wo = np.asarray(wo, np.float32)
    bf16 = ml_dtypes.bfloat16

    slopes = alibi_bias[0, :, 0, 1].copy()        # [H]; alibi[0,h,0,1] = slope_h

    inv_freq = 1.0 / (10000.0 ** (np.arange(0, HD, 2, dtype=np.float32) / HD))
    t = np.arange(S, dtype=np.float32)
    freqs = np.outer(t, inv_freq)                 # [S, 64]
    cos = np.cos(freqs).astype(np.float32).T      # [64, S]
    sin = np.sin(freqs).astype(np.float32).T
    cosF = np.ascontiguousarray(np.concatenate([cos, cos], 0)).astype(np.float16)
    sinF = np.ascontiguousarray(np.concatenate([-sin, sin], 0)).astype(np.float16)

    perm = np.concatenate([np.arange(0, HD, 2), np.arange(1, HD, 2)])
    p_ar = np.arange(128, dtype=np.float32)
    cmask = (p_ar[:, None] <= p_ar[None, :]).astype(bf16)
    ident = np.eye(128, dtype=np.float32).astype(bf16)
    ones = np.ones((128, 128), np.float32).astype(bf16)

    xTs = [np.ascontiguousarray(x[b].T).astype(np.float16) for b in range(B)]
    in_maps = []
    for core in range(8):
        b, g = divmod(core, KV)
        wq_g = wq[4 * g * HD:(4 * g + 4) * HD].reshape(NH, HD, D)[:, perm, :]
        wqT = np.ascontiguousarray(wq_g.reshape(NH * HD, D).T).astype(np.float16)
        wkT = np.ascontiguousarray(wk[g * HD:(g + 1) * HD][perm].T).astype(np.float16)
        wvT = np.ascontiguousarray(wv[g * HD:(g + 1) * HD].T).astype(np.float16)
        woT = np.ascontiguousarray(
            wo[:, 4 * g * HD:(4 * g + 4) * HD].T).astype(np.float16)
        biasb = np.zeros((128, NH * NBM), np.float32)
        for h in range(NH):
            sl = slopes[4 * g + h]
            for mi in range(NBM):
                m = mi - 3   # m = gridpos - kvtile; bias = slope*(j - i_gs - C)
                biasb[:, h * NBM + mi] = sl * (p_ar - 128.0 * m - EXP_C[h])
        in_maps.append({
            "xT": xTs[b], "wqT": wqT, "wkT": wkT, "wvT": wvT, "woT": woT,
            "cosF": cosF, "sinF": sinF, "biasb": biasb, "cmask": cmask,
            "ident": ident, "ones": ones,
        })
    return in_maps


def kernel(x, mask, alibi_bias, wq, wk, wv, wo, _trace=False, _trace_kwargs=None):
    nc = _get_nc()
    in_maps = _host_prep(x, alibi_bias, wq, wk, wv, wo)
    res = run_bass_kernel_spmd(nc, in_maps, list(range(8)), trace=_trace,
                               **(_trace_kwargs or {}))
    parts = [np.asarray(res.results[c]["part"], np.float32) for c in range(8)]
    out = np.stack([
        parts[0] + parts[1] + parts[2] + parts[3],
        parts[4] + parts[5] + parts[6] + parts[7],
    ]).astype(np.float32)
    if _trace:
        return out, res
    return out


# revision 15
# speedup vs baseline: 1.4998x; 1.0225x over previous
"""GQA attention (RoPE + ALiBi + causal) Bass kernel for Trainium2, 8 NeuronCores.

Sharding: core (b, g) = batch b in {0,1} x kv-group g in {0..3}; each core computes
its 4 query heads' attention for its batch and a partial output projection
(row-parallel wo); host sums the 4 group partials per batch.

Dataflow is 16-bit on the PE: fp16 for x/weights/Q/K/attn, bf16 for P and V
(bf16's fp32-size exponent carries the ALiBi recentering range). 16-bit
matmuls pay ~+32ns each over fp32r but halve PE power, which avoids most of
the chip's activity throttling (fp32r at full duty loses more to the 0.5x
util cap than the cadence penalty costs), and halve DMA traffic and SBUF.

  Phase 1 (per 512-q window): Q/K/V projections (6 PSUM accumulators over one
  streamed pass of xT). The first weight chunks and the window-0 x tiles go on
  the sync hwdge queue (earliest to start issuing); later weight chunks stream
  from gpsimd. PSUM->SBUF copies split across ACT/DVE (all-ACT for the last
  window so the DVE's RoPE backlog at the phase boundary is minimal),
  per-window RoPE (DVE) and V transpose (PE).
  Phase 2: per (window, head): scoresT = K^T Q (PSUM), P = exp(scale*scores
  + bias) with per-head-slot exp widths {128,256,256,512}: heads are ordered by
  descending ALiBi slope within each GQA group, so later slots (smaller slopes)
  tolerate coarser per-chunk bias recentering - fewer, wider ACT instructions.
  The -slope*q half of ALiBi cancels in softmax; the per-kv half plus the
  chunk-recentering constant comes from a host-built bias table indexed by
  (slot, chunkpos - kvtile). Diagonal chunks get a causal 0/1 mask multiply.
  Then outT += V_u^T P and den += ones^T P (PSUM accumulate); attn =
  outT * recip(den); each window's output-projection matmuls are interleaved
  as PE filler into the next window's attention loop, front-loaded onto the
  exp-heavy slot-0 head. Window 0 runs its heads cheapest-exp-first while the
  filler queue is still empty.
"""
import math
from contextlib import ExitStack

import numpy as np
import ml_dtypes

import concourse.bass as bass
import concourse.bacc as bacc
import concourse.tile as tile
from concourse import mybir
from concourse.bass_utils import run_bass_kernel_spmd

F32 = mybir.dt.float32
F16 = mybir.dt.float16
BF16 = mybir.dt.bfloat16

B, S, D = 2, 2048, 2048
H, KV, HD, REP = 16, 4, 128, 4
NH = 4                     # heads per core
NW = S // 512              # q-windows
ND = D // 128              # d_in tiles
NU = S // 128              # kv tiles
SCALE = 1.0 / math.sqrt(HD)

# per-head-slot exp chunk width and bias recentering constant; slot 0 holds the
# steepest ALiBi slope of the core's group (heads are slope-descending within a
# group), so it gets the finest recentering. Range check (worst slope s per
# slot, scores*scale ~ N(0,1), P in bf16 with fp32-range exponent):
#   exp arg in [score - s*C, score + s*(W-1-C)] -> within e^{+-87} for
#   (W,C,s) = (128,96,.707), (256,127,.5), (256,127,.354), (512,255,.25).
EXP_W = [128, 256, 256, 512]
EXP_C = [96.0, 127.0, 127.0, 255.0]
NBM = 19                   # bias cols per head slot: m = gridpos - kvtile in [-3, 15]
# rope-pair swap within each 32-partition quadrant (pairs live at +0/+16)
SHUF_MASK = list(range(16, 32)) + list(range(16))


def build():
    nc = bacc.Bacc(None)
    xT_d = nc.dram_tensor("xT", [D, S], F16, kind="ExternalInput")
    wq_d = nc.dram_tensor("wqT", [D, NH * HD], F16, kind="ExternalInput")
    wk_d = nc.dram_tensor("wkT", [D, HD], F16, kind="ExternalInput")
    wv_d = nc.dram_tensor("wvT", [D, HD], F16, kind="ExternalInput")
    wo_d = nc.dram_tensor("woT", [NH * HD, D], F16, kind="ExternalInput")
    cosF_d = nc.dram_tensor("cosF", [128, S], F16, kind="ExternalInput")
    sinF_d = nc.dram_tensor("sinF", [128, S], F16, kind="ExternalInput")
    biasb_d = nc.dram_tensor("biasb", [128, NH * NBM], F32, kind="ExternalInput")
    cmask_d = nc.dram_tensor("cmask", [128, 128], BF16, kind="ExternalInput")
    ident_d = nc.dram_tensor("ident", [128, 128], BF16, kind="ExternalInput")
    ones_d = nc.dram_tensor("ones", [128, 128], BF16, kind="ExternalInput")
    part_d = nc.dram_tensor("part", [S, D], F16, kind="ExternalOutput")

    PSUM = bass.MemorySpace.PSUM

    with tile.TileContext(nc) as tc:
        with ExitStack() as ctx:
            consts = ctx.enter_context(tc.tile_pool(name="consts", bufs=1))
            persist = ctx.enter_context(tc.tile_pool(name="persist", bufs=1))

            cosF = consts.tile([128, S], F16, tag="cosF")
            sinF = consts.tile([128, S], F16, tag="sinF")
            biasb = consts.tile([128, NH * NBM], F32, tag="biasb")
            cmask = consts.tile([128, 128], BF16, tag="cmask")
            ident = consts.tile([128, 128], BF16, tag="ident")
            ones = consts.tile([128, 128], BF16, tag="ones")

            qT = [persist.tile([128, S], F16, tag=f"qT{h}", name=f"qT{h}")
                  for h in range(NH)]
            kT = persist.tile([128, S], F16, tag="kT")
            vnat = persist.tile([128, S], BF16, tag="vnat")
            attn = [persist.tile([128, S], F16, tag=f"attn{h}", name=f"attn{h}")
                    for h in range(NH)]

            # ---------------- phase 1: Q/K/V projections (+ per-window RoPE) -----
            with tc.tile_pool(name="wqkv", bufs=1) as wpool, \
                 tc.tile_pool(name="xsl", bufs=10) as xpool, \
                 tc.tile_pool(name="vtmp", bufs=1) as vpool, \
                 tc.tile_pool(name="rope", bufs=3) as rp, \
                 tc.tile_pool(name="pps", bufs=1, space=PSUM) as pps, \
                 tc.tile_pool(name="tpp", bufs=1, space=PSUM) as tpp:
                wq_sb = wpool.tile([128, ND, NH * HD], F16, tag="wq")
                wk_sb = wpool.tile([128, ND, HD], F16, tag="wk")
                wv_sb = wpool.tile([128, ND, HD], F16, tag="wv")
                wqr = wq_d.rearrange("(t p) o -> p t o", p=128)
                wkr = wk_d.rearrange("(t p) o -> p t o", p=128)
                wvr = wv_d.rearrange("(t p) o -> p t o", p=128)

                xtiles = {}

                def issue_x(w, d):
                    xs = xpool.tile([128, 512], F16, tag="x", name="xs")
                    nc.sync.dma_start(
                        xs[:], xT_d[d * 128:(d + 1) * 128, w * 512:(w + 1) * 512])
                    xtiles[(w, d)] = xs

                # critical path on the sync hwdge queue (first to issue):
                # d0-1 weight chunks, then window-0 x tiles; remaining weight
                # chunks stream JIT from gpsimd; rope tables/consts follow the
                # w0 x prefetch (first needed ~25us in)
                d01 = slice(0, 2)
                nc.sync.dma_start(wk_sb[:, d01, :], wkr[:, d01, :])
                nc.sync.dma_start(wv_sb[:, d01, :], wvr[:, d01, :])
                nc.sync.dma_start(wq_sb[:, d01, :], wqr[:, d01, :])
                for g2 in range(1, ND // 2):
                    dsl = slice(2 * g2, 2 * g2 + 2)
                    nc.gpsimd.dma_start(wk_sb[:, dsl, :], wkr[:, dsl, :])
                    nc.gpsimd.dma_start(wv_sb[:, dsl, :], wvr[:, dsl, :])
                    nc.gpsimd.dma_start(wq_sb[:, dsl, :], wqr[:, dsl, :])
                for d in range(ND):
                    issue_x(0, d)
                nc.sync.dma_start(cosF[:], cosF_d[:])
                nc.sync.dma_start(sinF[:], sinF_d[:])
                nc.sync.dma_start(biasb[:], biasb_d[:])
                nc.sync.dma_start(cmask[:], cmask_d[:])
                nc.sync.dma_start(ident[:], ident_d[:])
                nc.sync.dma_start(ones[:], ones_d[:])
                vT = vpool.tile([128, S], BF16, tag="vT")

                for w in range(NW):
                    sl = slice(w * 512, (w + 1) * 512)
                    pq = [pps.tile([128, 512], F32, tag=f"pq{h}", name=f"pq{h}")
                          for h in range(NH)]
                    pk = pps.tile([128, 512], F32, tag="pk", name="pk")
                    pv = pps.tile([128, 512], F32, tag="pv", name="pv")
                    for d in range(ND):
                        if w > 0:
                            issue_x(w, d)
                        xs = xtiles.pop((w, d))
                        st, sp = (d == 0), (d == ND - 1)
                        nc.tensor.matmul(pk[:], wk_sb[:, d, :], xs[:], start=st, stop=sp)
                        nc.tensor.matmul(pv[:], wv_sb[:, d, :], xs[:], start=st, stop=sp)
                        for h in range(NH):
                            nc.tensor.matmul(pq[h][:], wq_sb[:, d, h * 128:(h + 1) * 128],
                                             xs[:], start=st, stop=sp)
                    # PSUM->SBUF copies split across ACT and DVE; the last
                    # window goes all-ACT so the DVE backlog at the phase
                    # boundary is only the rope itself
                    last = (w == NW - 1)
                    nc.scalar.copy(kT[:, sl], pk[:])
                    nc.scalar.copy(qT[0][:, sl], pq[0][:])
                    nc.scalar.copy(qT[1][:, sl], pq[1][:])
                    if last:
                        nc.scalar.copy(vT[:, sl], pv[:])
                        nc.scalar.copy(qT[2][:, sl], pq[2][:])
                        nc.scalar.copy(qT[3][:, sl], pq[3][:])
                    else:
                        nc.vector.tensor_copy(vT[:, sl], pv[:])
                        nc.vector.tensor_copy(qT[2][:, sl], pq[2][:])
                        nc.vector.tensor_copy(qT[3][:, sl], pq[3][:])

                    # RoPE on this window's q/k slices: out = cosF*z +
                    # sinF*swap(z); rope pairs are laid out within 32-partition
                    # quadrants (host perm) so the swap is a DVE stream_shuffle
                    for tgt in [kT] + qT:
                        qb = rp.tile([128, 512], F16, tag="qb", name="qb")
                        nc.vector.stream_shuffle(qb[:], tgt[:, sl], SHUF_MASK)
                        t1 = rp.tile([128, 512], F16, tag="t1", name="t1")
                        nc.vector.tensor_mul(t1[:], tgt[:, sl], cosF[:, sl])
                        nc.vector.tensor_mul(qb[:], qb[:], sinF[:, sl])
                        nc.vector.tensor_add(tgt[:, sl], t1[:], qb[:])

                    # V transpose for this window's 4 kv tiles:
                    # vT [hd, s] -> vnat [s(part), hd]
                    for u in range(4 * w, 4 * w + 4):
                        tp = tpp.tile([128, 128], BF16, tag=f"tp{u % 2}",
                                      name=f"tp{u}")
                        nc.tensor.transpose(tp[:], vT[:, u * 128:(u + 1) * 128],
                                            ident[:])
                        nc.scalar.copy(vnat[:, u * 128:(u + 1) * 128], tp[:])

            # ---------------- phase 2: attention + output projection ------------
            with tc.tile_pool(name="sp", bufs=3, space=PSUM) as sp, \
                 tc.tile_pool(name="dp", bufs=1, space=PSUM) as dp, \
                 tc.tile_pool(name="op", bufs=2, space=PSUM) as op, \
                 tc.tile_pool(name="ojp", bufs=2, space=PSUM) as ojp, \
                 tc.tile_pool(name="Pp", bufs=8) as Pp, \
                 tc.tile_pool(name="ep", bufs=4) as ep, \
                 tc.tile_pool(name="wop", bufs=1) as wop, \
                 tc.tile_pool(name="ostg", bufs=6) as ostg:
                wo_sb = wop.tile([128, NH, D], F16, tag="wo")
                nc.gpsimd.dma_start(wo_sb[:], wo_d.rearrange("(h p) o -> p h o", p=128))

                filler_q = []

                def emit_fillers(n, eng="v"):
                    for _ in range(n):
                        if not filler_q:
                            return
                        filler_q.pop(0)(eng)

                def make_unit(w_, mq_, dwin_):
                    def unit(eng):
                        m_ = 4 * w_ + mq_
                        po = ojp.tile([128, 512], F32, tag="oj",
                                      name=f"po{m_}_{dwin_}")
                        for h_ in range(NH):
                            nc.tensor.matmul(
                                po[:],
                                attn[h_][:, m_ * 128:(m_ + 1) * 128],
                                wo_sb[:, h_, dwin_ * 512:(dwin_ + 1) * 512],
                                start=(h_ == 0), stop=(h_ == NH - 1))
                        so = ostg.tile([128, 512], F16, tag="so", name="so")
                        if eng == "v":
                            nc.vector.tensor_copy(so[:], po[:])
                        else:
                            nc.scalar.copy(so[:], po[:])
                        nc.sync.dma_start(
                            part_d[m_ * 128:(m_ + 1) * 128,
                                   dwin_ * 512:(dwin_ + 1) * 512], so[:])
                    return unit

                ucount = 0
                for w in range(NW):
                    qsl = slice(w * 512, (w + 1) * 512)
                    U = 4 * (w + 1)
                    # window 0 runs cheapest-exp heads first (no filler supply
                    # yet, so let ACT race ahead of the PE)
                    horder = [3, 2, 1, 0] if w == 0 else [0, 1, 2, 3]
                    for h in horder:
                        o_ps = op.tile([128, 512], F32, tag="o", name=f"o{w}_{h}")
                        d_ps = dp.tile([128, 512], F32, tag="den", name=f"d{w}_{h}")
                        pend = None
                        for u in range(U):
                            i0 = max(0, u - 4 * w)
                            n0 = 128 * i0
                            s_ps = sp.tile([128, 512], F32, tag="s", name="s")
                            nc.tensor.matmul(
                                s_ps[:, n0:512],
                                kT[:, u * 128:(u + 1) * 128],
                                qT[h][:, w * 512 + n0:(w + 1) * 512],
                                start=True, stop=True)
                            Pt = Pp.tile([128, 512], BF16, tag="P", name="P")
                            # exp pieces on a fixed per-slot q-grid (grid
                            # position independent of u so the dropped per-q
                            # ALiBi term is consistent across kv tiles);
                            # diagonal tiles clip the first piece at n0.
                            # bias col m = grid position (128-units) - kv tile
                            for gs in range(0, 512, EXP_W[h]):
                                pcs = max(gs, n0)
                                pce = gs + EXP_W[h]
                                if pcs >= pce:
                                    continue
                                m = 4 * w + gs // 128 - u + 3
                                nc.scalar.activation(
                                    Pt[:, pcs:pce],
                                    s_ps[:, pcs:pce],
                                    mybir.ActivationFunctionType.Exp,
                                    bias=biasb[:, h * NBM + m:h * NBM + m + 1],
                                    scale=SCALE)
                            if u >= 4 * w:
                                # diagonal kv tile: causal 0/1 mask on its block
                                nc.vector.tensor_mul(Pt[:, n0:n0 + 128],
                                                     Pt[:, n0:n0 + 128], cmask[:])
                            ucount += 1
                            # front-load PE filler onto the exp-heavy slot-0
                            # head; later slots are self-paced
                            if h == 0 or ucount % 2 == 0:
                                emit_fillers(1)
                            if pend is not None:
                                pPt, pn0, pu = pend
                                nc.tensor.matmul(o_ps[:, pn0:512],
                                                 vnat[:, pu * 128:(pu + 1) * 128],
                                                 pPt[:, pn0:512],
                                                 start=(pu == 0), stop=False)
                                nc.tensor.matmul(d_ps[:, pn0:512], ones[:],
                                                 pPt[:, pn0:512],
                                                 start=(pu == 0), stop=False)
                            pend = (Pt, n0, u)
                        pPt, pn0, pu = pend
                        nc.tensor.matmul(o_ps[:, pn0:512],
                                         vnat[:, pu * 128:(pu + 1) * 128],
                                         pPt[:, pn0:512], start=(pu == 0), stop=True)
                        nc.tensor.matmul(d_ps[:, pn0:512], ones[:],
                                         pPt[:, pn0:512], start=(pu == 0), stop=True)
                        rec = ep.tile([128, 512], F32, tag="rec", name="rec")
                        nc.vector.reciprocal_approx_fast(rec[:], d_ps[:])
                        nc.vector.tensor_mul(attn[h][:, qsl], o_ps[:], rec[:])

                    # enqueue this window's output projection as PE filler
                    # for the next window's attention loop
                    for mq in range(4):
                        for dwin in range(4):
                            filler_q.append(make_unit(w, mq, dwin))

                # drain remaining units; their staging copies go on ACT, which
                # is idle once the exps are done
                emit_fillers(len(filler_q), eng="s")
    nc.finalize()
    return nc


_NC_CACHE = {}


def _get_nc():
    if "nc" not in _NC_CACHE:
        _NC_CACHE["nc"] = build()
    return _NC_CACHE["nc"]


def _host_prep(x, alibi_bias, wq, wk, wv, wo):
    """Build per-core input maps (shard + transpose + rope tables + bias tables)."""
    x = np.asarray(x, np.float32)
    alibi_bias = np.asarray(alibi_bias, np.float32)
    wq = np.asarray(wq, np.float32)
    wk = np.asarray(wk, np.float32)
    wv = np.asarray(wv, np.float32)
    wo = np.asarray(wo, np.float32)
    bf16 = ml_dtypes.bfloat16

    slopes = alibi_bias[0, :, 0, 1].copy()        # [H]; alibi[0,h,0,1] = slope_h

    inv_freq = 1.0 / (10000.0 ** (np.arange(0, HD, 2, dtype=np.float32) / HD))
    t = np.arange(S, dtype=np.float32)
    freqs = np.outer(t, inv_freq)                 # [S, 64]
    cos = np.cos(freqs).astype(np.float32).T      # [64, S]
    sin = np.sin(freqs).astype(np.float32).T
    # quadrant-paired rope layout: pair i -> quadrant i//16, slots j and j+16
    # (even element at row 32*(i//16)+i%16, odd at +16); the on-device swap is
    # then a within-quadrant stream_shuffle
    perm = np.zeros(HD, np.int64)
    row_f = np.zeros(HD, np.int64)    # freq index per row
    row_sg = np.zeros(HD, np.float32)  # sin sign per row
    for i in range(64):
        qd, j = divmod(i, 16)
        perm[32 * qd + j] = 2 * i
        perm[32 * qd + 16 + j] = 2 * i + 1
        row_f[32 * qd + j] = i
        row_f[32 * qd + 16 + j] = i
        row_sg[32 * qd + j] = -1.0
        row_sg[32 * qd + 16 + j] = 1.0
    cosF = np.ascontiguousarray(cos[row_f]).astype(np.float16)
    sinF = np.ascontiguousarray(sin[row_f] * row_sg[:, None]).astype(np.float16)
    p_ar = np.arange(128, dtype=np.float32)
    cmask = (p_ar[:, None] <= p_ar[None, :]).astype(bf16)
    ident = np.eye(128, dtype=np.float32).astype(bf16)
    ones = np.ones((128, 128), np.float32).astype(bf16)

    xTs = [np.ascontiguousarray(x[b].T).astype(np.float16) for b in range(B)]
    in_maps = []
    for core in range(8):
        b, g = divmod(core, KV)
        wq_g = wq[4 * g * HD:(4 * g + 4) * HD].reshape(NH, HD, D)[:, perm, :]
        wqT = np.ascontiguousarray(wq_g.reshape(NH * HD, D).T).astype(np.float16)
        wkT = np.ascontiguousarray(wk[g * HD:(g + 1) * HD][perm].T).astype(np.float16)
        wvT = np.ascontiguousarray(wv[g * HD:(g + 1) * HD].T).astype(np.float16)
        woT = np.ascontiguousarray(
            wo[:, 4 * g * HD:(4 * g + 4) * HD].T).astype(np.float16)
        biasb = np.zeros((128, NH * NBM), np.float32)
        for h in range(NH):
            sl = slopes[4 * g + h]
            for mi in range(NBM):
                m = mi - 3   # m = gridpos - kvtile; bias = slope*(j - i_gs - C)
                biasb[:, h * NBM + mi] = sl * (p_ar - 128.0 * m - EXP_C[h])
        in_maps.append({
            "xT": xTs[b], "wqT": wqT, "wkT": wkT, "wvT": wvT, "woT": woT,
            "cosF": cosF, "sinF": sinF, "biasb": biasb, "cmask": cmask,
            "ident": ident, "ones": ones,
        })
    return in_maps


def kernel(x, mask, alibi_bias, wq, wk, wv, wo, _trace=False, _trace_kwargs=None):
    nc = _get_nc()
    in_maps = _host_prep(x, alibi_bias, wq, wk, wv, wo)
    res = run_bass_kernel_spmd(nc, in_maps, list(range(8)), trace=_trace,
                               **(_trace_kwargs or {}))
    parts = [np.asarray(res.results[c]["part"], np.float32) for c in range(8)]
    out = np.stack([
        parts[0] + parts[1] + parts[2] + parts[3],
        parts[4] + parts[5] + parts[6] + parts[7],
    ]).astype(np.float32)
    if _trace:
        return out, res
    return out


# revision 16
# speedup vs baseline: 1.5032x; 1.0023x over previous
"""GQA attention (RoPE + ALiBi + causal) Bass kernel for Trainium2, 8 NeuronCores.

Sharding: core (b, g) = batch b in {0,1} x kv-group g in {0..3}; each core computes
its 4 query heads' attention for its batch and a partial output projection
(row-parallel wo); host sums the 4 group partials per batch.

Dataflow is 16-bit on the PE: fp16 for x/weights/Q/K/attn, bf16 for P and V
(bf16's fp32-size exponent carries the ALiBi recentering range). 16-bit
matmuls pay ~+32ns each over fp32r but halve PE power, which avoids most of
the chip's activity throttling (fp32r at full duty loses more to the 0.5x
util cap than the cadence penalty costs), and halve DMA traffic and SBUF.

  Phase 1 (per 512-q window): Q/K/V projections (6 PSUM accumulators over one
  streamed pass of xT). The first weight chunks and the window-0 x tiles go on
  the sync hwdge queue (earliest to start issuing); later weight chunks stream
  from gpsimd. PSUM->SBUF copies split across ACT/DVE (all-ACT for the last
  window so the DVE's RoPE backlog at the phase boundary is minimal),
  per-window RoPE (DVE) and V transpose (PE).
  Phase 2: per (window, head): scoresT = K^T Q (PSUM), P = exp(scale*scores
  + bias) with per-head-slot exp widths {128,256,256,512}: heads are ordered by
  descending ALiBi slope within each GQA group, so later slots (smaller slopes)
  tolerate coarser per-chunk bias recentering - fewer, wider ACT instructions.
  The -slope*q half of ALiBi cancels in softmax; the per-kv half plus the
  chunk-recentering constant comes from a host-built bias table indexed by
  (slot, chunkpos - kvtile). Diagonal chunks get a causal 0/1 mask multiply.
  Then outT += V_u^T P and den += ones^T P (PSUM accumulate); attn =
  outT * recip(den); each window's output-projection matmuls are interleaved
  as PE filler into the next window's attention loop, front-loaded onto the
  exp-heavy slot-0 head. Window 0 runs its heads cheapest-exp-first while the
  filler queue is still empty.
"""
import math
from contextlib import ExitStack

import numpy as np
import ml_dtypes

import concourse.bass as bass
import concourse.bacc as bacc
import concourse.tile as tile
from concourse import mybir
from concourse.bass_utils import run_bass_kernel_spmd

F32 = mybir.dt.float32
F16 = mybir.dt.float16
BF16 = mybir.dt.bfloat16

B, S, D = 2, 2048, 2048
H, KV, HD, REP = 16, 4, 128, 4
NH = 4                     # heads per core
NW = S // 512              # q-windows
ND = D // 128              # d_in tiles
NU = S // 128              # kv tiles
SCALE = 1.0 / math.sqrt(HD)

# per-head-slot exp chunk width and bias recentering constant; slot 0 holds the
# steepest ALiBi slope of the core's group (heads are slope-descending within a
# group), so it gets the finest recentering. Range check (worst slope s per
# slot, scores*scale ~ N(0,1), P in bf16 with fp32-range exponent):
#   exp arg in [score - s*C, score + s*(W-1-C)] -> within e^{+-87} for
#   (W,C,s) = (128,96,.707), (256,127,.5), (256,127,.354), (512,255,.25).
EXP_W = [128, 256, 256, 512]
EXP_C = [96.0, 127.0, 127.0, 255.0]
NBM = 19                   # bias cols per head slot: m = gridpos - kvtile in [-3, 15]
# rope-pair swap within each 32-partition quadrant (pairs live at +0/+16)
SHUF_MASK = list(range(16, 32)) + list(range(16))


def build():
    nc = bacc.Bacc(None)
    xT_d = nc.dram_tensor("xT", [D, S], F16, kind="ExternalInput")
    wq_d = nc.dram_tensor("wqT", [D, NH * HD], F16, kind="ExternalInput")
    wk_d = nc.dram_tensor("wkT", [D, HD], F16, kind="ExternalInput")
    wv_d = nc.dram_tensor("wvT", [D, HD], F16, kind="ExternalInput")
    wo_d = nc.dram_tensor("woT", [NH * HD, D], F16, kind="ExternalInput")
    cosF_d = nc.dram_tensor("cosF", [128, S], F16, kind="ExternalInput")
    sinF_d = nc.dram_tensor("sinF", [128, S], F16, kind="ExternalInput")
    biasb_d = nc.dram_tensor("biasb", [128, NH * NBM], F32, kind="ExternalInput")
    cmask_d = nc.dram_tensor("cmask", [128, 128], BF16, kind="ExternalInput")
    ident_d = nc.dram_tensor("ident", [128, 128], BF16, kind="ExternalInput")
    ones_d = nc.dram_tensor("ones", [128, 128], BF16, kind="ExternalInput")
    part_d = nc.dram_tensor("part", [S, D], F16, kind="ExternalOutput")

    PSUM = bass.MemorySpace.PSUM

    with tile.TileContext(nc) as tc:
        with ExitStack() as ctx:
            consts = ctx.enter_context(tc.tile_pool(name="consts", bufs=1))
            persist = ctx.enter_context(tc.tile_pool(name="persist", bufs=1))

            cosF = consts.tile([128, S], F16, tag="cosF")
            sinF = consts.tile([128, S], F16, tag="sinF")
            biasb = consts.tile([128, NH * NBM], F32, tag="biasb")
            cmask = consts.tile([128, 128], BF16, tag="cmask")
            ident = consts.tile([128, 128], BF16, tag="ident")
            ones = consts.tile([128, 128], BF16, tag="ones")

            qT = [persist.tile([128, S], F16, tag=f"qT{h}", name=f"qT{h}")
                  for h in range(NH)]
            kT = persist.tile([128, S], F16, tag="kT")
            vnat = persist.tile([128, S], BF16, tag="vnat")
            attn = [persist.tile([128, S], F16, tag=f"attn{h}", name=f"attn{h}")
                    for h in range(NH)]

            # ---------------- phase 1: Q/K/V projections (+ per-window RoPE) -----
            with tc.tile_pool(name="wqkv", bufs=1) as wpool, \
                 tc.tile_pool(name="xsl", bufs=10) as xpool, \
                 tc.tile_pool(name="vtmp", bufs=1) as vpool, \
                 tc.tile_pool(name="rope", bufs=3) as rp, \
                 tc.tile_pool(name="pps", bufs=1, space=PSUM) as pps, \
                 tc.tile_pool(name="tpp", bufs=1, space=PSUM) as tpp:
                wq_sb = wpool.tile([128, ND, NH * HD], F16, tag="wq")
                wk_sb = wpool.tile([128, ND, HD], F16, tag="wk")
                wv_sb = wpool.tile([128, ND, HD], F16, tag="wv")
                wqr = wq_d.rearrange("(t p) o -> p t o", p=128)
                wkr = wk_d.rearrange("(t p) o -> p t o", p=128)
                wvr = wv_d.rearrange("(t p) o -> p t o", p=128)

                xtiles = {}

                def issue_x(w, d):
                    xs = xpool.tile([128, 512], F16, tag="x", name="xs")
                    nc.sync.dma_start(
                        xs[:], xT_d[d * 128:(d + 1) * 128, w * 512:(w + 1) * 512])
                    xtiles[(w, d)] = xs

                # critical path on the sync hwdge queue (first to issue):
                # x00, then d0-1 weight chunks, then the rest of window-0's x;
                # remaining weight chunks stream JIT from gpsimd, followed by
                # the rope tables/consts (first needed ~25us in)
                issue_x(0, 0)
                d01 = slice(0, 2)
                nc.sync.dma_start(wk_sb[:, d01, :], wkr[:, d01, :])
                nc.sync.dma_start(wv_sb[:, d01, :], wvr[:, d01, :])
                nc.sync.dma_start(wq_sb[:, d01, :], wqr[:, d01, :])
                for g2 in range(1, ND // 2):
                    dsl = slice(2 * g2, 2 * g2 + 2)
                    nc.gpsimd.dma_start(wk_sb[:, dsl, :], wkr[:, dsl, :])
                    nc.gpsimd.dma_start(wv_sb[:, dsl, :], wvr[:, dsl, :])
                    nc.gpsimd.dma_start(wq_sb[:, dsl, :], wqr[:, dsl, :])
                for d in range(1, ND):
                    issue_x(0, d)
                nc.gpsimd.dma_start(cosF[:], cosF_d[:])
                nc.gpsimd.dma_start(sinF[:], sinF_d[:])
                nc.gpsimd.dma_start(biasb[:], biasb_d[:])
                nc.gpsimd.dma_start(cmask[:], cmask_d[:])
                nc.gpsimd.dma_start(ident[:], ident_d[:])
                nc.gpsimd.dma_start(ones[:], ones_d[:])
                vT = vpool.tile([128, S], BF16, tag="vT")

                for w in range(NW):
                    sl = slice(w * 512, (w + 1) * 512)
                    pq = [pps.tile([128, 512], F32, tag=f"pq{h}", name=f"pq{h}")
                          for h in range(NH)]
                    pk = pps.tile([128, 512], F32, tag="pk", name="pk")
                    pv = pps.tile([128, 512], F32, tag="pv", name="pv")
                    for d in range(ND):
                        if w > 0:
                            issue_x(w, d)
                        xs = xtiles.pop((w, d))
                        st, sp = (d == 0), (d == ND - 1)
                        nc.tensor.matmul(pk[:], wk_sb[:, d, :], xs[:], start=st, stop=sp)
                        nc.tensor.matmul(pv[:], wv_sb[:, d, :], xs[:], start=st, stop=sp)
                        for h in range(NH):
                            nc.tensor.matmul(pq[h][:], wq_sb[:, d, h * 128:(h + 1) * 128],
                                             xs[:], start=st, stop=sp)
                    # PSUM->SBUF copies split across ACT and DVE; the last
                    # window goes all-ACT so the DVE backlog at the phase
                    # boundary is only the rope itself
                    last = (w == NW - 1)
                    nc.scalar.copy(kT[:, sl], pk[:])
                    nc.scalar.copy(qT[0][:, sl], pq[0][:])
                    nc.scalar.copy(qT[1][:, sl], pq[1][:])
                    if last:
                        nc.scalar.copy(vT[:, sl], pv[:])
                        nc.scalar.copy(qT[2][:, sl], pq[2][:])
                        nc.scalar.copy(qT[3][:, sl], pq[3][:])
                    else:
                        nc.vector.tensor_copy(vT[:, sl], pv[:])
                        nc.vector.tensor_copy(qT[2][:, sl], pq[2][:])
                        nc.vector.tensor_copy(qT[3][:, sl], pq[3][:])

                    # RoPE on this window's q/k slices: out = cosF*z +
                    # sinF*swap(z); rope pairs are laid out within 32-partition
                    # quadrants (host perm) so the swap is a DVE stream_shuffle
                    for tgt in [kT] + qT:
                        qb = rp.tile([128, 512], F16, tag="qb", name="qb")
                        nc.vector.stream_shuffle(qb[:], tgt[:, sl], SHUF_MASK)
                        t1 = rp.tile([128, 512], F16, tag="t1", name="t1")
                        nc.vector.tensor_mul(t1[:], tgt[:, sl], cosF[:, sl])
                        nc.vector.tensor_mul(qb[:], qb[:], sinF[:, sl])
                        nc.vector.tensor_add(tgt[:, sl], t1[:], qb[:])

                    # V transpose for this window's 4 kv tiles:
                    # vT [hd, s] -> vnat [s(part), hd]
                    for u in range(4 * w, 4 * w + 4):
                        tp = tpp.tile([128, 128], BF16, tag=f"tp{u % 2}",
                                      name=f"tp{u}")
                        nc.tensor.transpose(tp[:], vT[:, u * 128:(u + 1) * 128],
                                            ident[:])
                        nc.scalar.copy(vnat[:, u * 128:(u + 1) * 128], tp[:])

            # ---------------- phase 2: attention + output projection ------------
            with tc.tile_pool(name="sp", bufs=3, space=PSUM) as sp, \
                 tc.tile_pool(name="dp", bufs=1, space=PSUM) as dp, \
                 tc.tile_pool(name="op", bufs=2, space=PSUM) as op, \
                 tc.tile_pool(name="ojp", bufs=2, space=PSUM) as ojp, \
                 tc.tile_pool(name="Pp", bufs=8) as Pp, \
                 tc.tile_pool(name="ep", bufs=4) as ep, \
                 tc.tile_pool(name="wop", bufs=1) as wop, \
                 tc.tile_pool(name="ostg", bufs=6) as ostg:
                wo_sb = wop.tile([128, NH, D], F16, tag="wo")
                nc.gpsimd.dma_start(wo_sb[:], wo_d.rearrange("(h p) o -> p h o", p=128))

                filler_q = []

                def emit_fillers(n, eng="v"):
                    for _ in range(n):
                        if not filler_q:
                            return
                        filler_q.pop(0)(eng)

                def make_unit(w_, mq_, dwin_):
                    def unit(eng):
                        m_ = 4 * w_ + mq_
                        po = ojp.tile([128, 512], F32, tag="oj",
                                      name=f"po{m_}_{dwin_}")
                        for h_ in range(NH):
                            nc.tensor.matmul(
                                po[:],
                                attn[h_][:, m_ * 128:(m_ + 1) * 128],
                                wo_sb[:, h_, dwin_ * 512:(dwin_ + 1) * 512],
                                start=(h_ == 0), stop=(h_ == NH - 1))
                        so = ostg.tile([128, 512], F16, tag="so", name="so")
                        if eng == "v":
                            nc.vector.tensor_copy(so[:], po[:])
                        else:
                            nc.scalar.copy(so[:], po[:])
                        nc.sync.dma_start(
                            part_d[m_ * 128:(m_ + 1) * 128,
                                   dwin_ * 512:(dwin_ + 1) * 512], so[:])
                    return unit

                ucount = 0
                for w in range(NW):
                    qsl = slice(w * 512, (w + 1) * 512)
                    U = 4 * (w + 1)
                    # window 0 runs cheapest-exp heads first (no filler supply
                    # yet, so let ACT race ahead of the PE)
                    horder = [3, 2, 1, 0] if w == 0 else [0, 1, 2, 3]
                    for h in horder:
                        o_ps = op.tile([128, 512], F32, tag="o", name=f"o{w}_{h}")
                        d_ps = dp.tile([128, 512], F32, tag="den", name=f"d{w}_{h}")
                        pend = None
                        for u in range(U):
                            i0 = max(0, u - 4 * w)
                            n0 = 128 * i0
                            s_ps = sp.tile([128, 512], F32, tag="s", name="s")
                            nc.tensor.matmul(
                                s_ps[:, n0:512],
                                kT[:, u * 128:(u + 1) * 128],
                                qT[h][:, w * 512 + n0:(w + 1) * 512],
                                start=True, stop=True)
                            Pt = Pp.tile([128, 512], BF16, tag="P", name="P")
                            # exp pieces on a fixed per-slot q-grid (grid
                            # position independent of u so the dropped per-q
                            # ALiBi term is consistent across kv tiles);
                            # diagonal tiles clip the first piece at n0.
                            # bias col m = grid position (128-units) - kv tile
                            for gs in range(0, 512, EXP_W[h]):
                                pcs = max(gs, n0)
                                pce = gs + EXP_W[h]
                                if pcs >= pce:
                                    continue
                                m = 4 * w + gs // 128 - u + 3
                                nc.scalar.activation(
                                    Pt[:, pcs:pce],
                                    s_ps[:, pcs:pce],
                                    mybir.ActivationFunctionType.Exp,
                                    bias=biasb[:, h * NBM + m:h * NBM + m + 1],
                                    scale=SCALE)
                            if u >= 4 * w:
                                # diagonal kv tile: causal 0/1 mask on its block
                                nc.vector.tensor_mul(Pt[:, n0:n0 + 128],
                                                     Pt[:, n0:n0 + 128], cmask[:])
                            ucount += 1
                            # front-load PE filler onto the exp-heavy slot-0
                            # head; later slots are self-paced
                            if h == 0 or ucount % 2 == 0:
                                emit_fillers(1)
                            if pend is not None:
                                pPt, pn0, pu = pend
                                nc.tensor.matmul(o_ps[:, pn0:512],
                                                 vnat[:, pu * 128:(pu + 1) * 128],
                                                 pPt[:, pn0:512],
                                                 start=(pu == 0), stop=False)
                                nc.tensor.matmul(d_ps[:, pn0:512], ones[:],
                                                 pPt[:, pn0:512],
                                                 start=(pu == 0), stop=False)
                            pend = (Pt, n0, u)
                        pPt, pn0, pu = pend
                        nc.tensor.matmul(o_ps[:, pn0:512],
                                         vnat[:, pu * 128:(pu + 1) * 128],
                                         pPt[:, pn0:512], start=(pu == 0), stop=True)
                        nc.tensor.matmul(d_ps[:, pn0:512], ones[:],
                                         pPt[:, pn0:512], start=(pu == 0), stop=True)
                        rec = ep.tile([128, 512], F32, tag="rec", name="rec")
                        nc.vector.reciprocal_approx_fast(rec[:], d_ps[:])
                        nc.vector.tensor_mul(attn[h][:, qsl], o_ps[:], rec[:])

                    # enqueue this window's output projection as PE filler
                    # for the next window's attention loop
                    for mq in range(4):
                        for dwin in range(4):
                            filler_q.append(make_unit(w, mq, dwin))

                # drain remaining units; their staging copies go on ACT, which
                # is idle once the exps are done
                emit_fillers(len(filler_q), eng="s")
    nc.finalize()
    return nc


_NC_CACHE = {}


def _get_nc():
    if "nc" not in _NC_CACHE:
        _NC_CACHE["nc"] = build()
    return _NC_CACHE["nc"]


def _host_prep(x, alibi_bias, wq, wk, wv, wo):
    """Build per-core input maps (shard + transpose + rope tables + bias tables)."""
    x = np.asarray(x, np.float32)
    alibi_bias = np.asarray(alibi_bias, np.float32)
    wq = np.asarray(wq, np.float32)
    wk = np.asarray(wk, np.float32)
    wv = np.asarray(wv, np.float32)
    wo = np.asarray(wo, np.float32)
    bf16 = ml_dtypes.bfloat16

    slopes = alibi_bias[0, :, 0, 1].copy()        # [H]; alibi[0,h,0,1] = slope_h

    inv_freq = 1.0 / (10000.0 ** (np.arange(0, HD, 2, dtype=np.float32) / HD))
    t = np.arange(S, dtype=np.float32)
    freqs = np.outer(t, inv_freq)                 # [S, 64]
    cos = np.cos(freqs).astype(np.float32).T      # [64, S]
    sin = np.sin(freqs).astype(np.float32).T
    # quadrant-paired rope layout: pair i -> quadrant i//16, slots j and j+16
    # (even element at row 32*(i//16)+i%16, odd at +16); the on-device swap is
    # then a within-quadrant stream_shuffle
    perm = np.zeros(HD, np.int64)
    row_f = np.zeros(HD, np.int64)    # freq index per row
    row_sg = np.zeros(HD, np.float32)  # sin sign per row
    for i in range(64):
        qd, j = divmod(i, 16)
        perm[32 * qd + j] = 2 * i
        perm[32 * qd + 16 + j] = 2 * i + 1
        row_f[32 * qd + j] = i
        row_f[32 * qd + 16 + j] = i
        row_sg[32 * qd + j] = -1.0
        row_sg[32 * qd + 16 + j] = 1.0
    cosF = np.ascontiguousarray(cos[row_f]).astype(np.float16)
    sinF = np.ascontiguousarray(sin[row_f] * row_sg[:, None]).astype(np.float16)
    p_ar = np.arange(128, dtype=np.float32)
    cmask = (p_ar[:, None] <= p_ar[None, :]).astype(bf16)
    ident = np.eye(128, dtype=np.float32).astype(bf16)
    ones = np.ones((128, 128), np.float32).astype(bf16)

    xTs = [np.ascontiguousarray(x[b].T).astype(np.float16) for b in range(B)]
    in_maps = []
    for core in range(8):
        b, g = divmod(core, KV)
        wq_g = wq[4 * g * HD:(4 * g + 4) * HD].reshape(NH, HD, D)[:, perm, :]
        wqT = np.ascontiguousarray(wq_g.reshape(NH * HD, D).T).astype(np.float16)
        wkT = np.ascontiguousarray(wk[g * HD:(g + 1) * HD][perm].T).astype(np.float16)
        wvT = np.ascontiguousarray(wv[g * HD:(g + 1) * HD].T).astype(np.float16)
        woT = np.ascontiguousarray(
            wo[:, 4 * g * HD:(4 * g + 4) * HD].T).astype(np.float16)
        biasb = np.zeros((128, NH * NBM), np.float32)
        for h in range(NH):
            sl = slopes[4 * g + h]
            for mi in range(NBM):
                m = mi - 3   # m = gridpos - kvtile; bias = slope*(j - i_gs - C)
                biasb[:, h * NBM + mi] = sl * (p_ar - 128.0 * m - EXP_C[h])
        in_maps.append({
            "xT": xTs[b], "wqT": wqT, "wkT": wkT, "wvT": wvT, "woT": woT,
            "cosF": cosF, "sinF": sinF, "biasb": biasb, "cmask": cmask,
            "ident": ident, "ones": ones,
        })
    return in_maps


def kernel(x, mask, alibi_bias, wq, wk, wv, wo, _trace=False, _trace_kwargs=None):
    nc = _get_nc()
    in_maps = _host_prep(x, alibi_bias, wq, wk, wv, wo)
    res = run_bass_kernel_spmd(nc, in_maps, list(range(8)), trace=_trace,
                               **(_trace_kwargs or {}))
    parts = [np.asarray(res.results[c]["part"], np.float32) for c in range(8)]
    out = np.stack([
        parts[0] + parts[1] + parts[2] + parts[3],
        parts[4] + parts[5] + parts[6] + parts[7],
    ]).astype(np.float32)
    if _trace:
        return out, res
    return out
